# revision 67
# baseline (speedup 1.0000x reference)
"""AbsoluteAttention Trainium2 kernel — 8-core SPMD, p2p state exchange.

Math: the reference's B*T*T*H `scores` tensor is low rank:
    scores[b,t,l,h] = qsum[b,t,h] * (time_q[t,h,:] . time_k[l,h,:])
so
    loading[b,t,h,:] = qsum[b,t,h] * TQ[t,h,:] @ (TK[:,h,:]^T @ kv[b,:,h,:])
which reduces the attention to a per-(batch,head) 64x64 "state" that is the
only cross-row coupling.  Data-parallel over the 4096 rows of flattened
(B,T): 512 rows per core; the only cross-core traffic is the [64, 768]
per-batch state, reduced across each batch's 4 cores.

The state reduction is done with direct SBUF->SBUF remote_dma_broadcast
(XOR-relative peers d=1,2,3 within each aligned 4-group) instead of a
collective: each core sends its partial and sums the 4 copies locally.
A bir-kernel barrier (prelude AllGather) makes the peer SBUF writes safe;
the barrier/remote-sem waits are spliced in AFTER tile scheduling because
the tile scheduler's single-core sim cannot see remote increments.

State is accumulated directly in the exchange layout [128, 384]:
    state[64g+j, 64kt+d] = state_head(kt+6g)[j, d]
via tile_position=(0, 64g) on the per-head state matmuls, so no
rearrangement is needed between PSUM, the wire, and the loadT stationary.
Host-side, Wo^T rows / time_q rows are permuted to the same head order.

Matmuls run fp8e4 DoubleRow (weights x32, loadT x16); residual/output in
bf16 (layernorm is scale-invariant; eps is pre-scaled).
"""
import os
import sys

for _p in ("/opt/trn_rl_repo", "/root/.axon_site/_ro/trn_rl_repo"):
    if os.path.isdir(_p) and _p not in sys.path:
        sys.path.insert(0, _p)

import numpy as np
import ml_dtypes

# Persistent executable cache: lets a fresh process skip the multi-minute
# compile if this kernel was compiled on this machine before.
try:
    import jax as _jax
    _jax.config.update("jax_compilation_cache_dir",
                       os.path.expanduser("~/.cache/absatt_jax_cache"))
    _jax.config.update("jax_persistent_cache_min_compile_time_secs", 1.0)
    _jax.config.update("jax_persistent_cache_min_entry_size_bytes", 0)
except Exception:
    pass

import concourse.bass as bass
import concourse.bacc as bacc
import concourse.tile as tile
import concourse.mybir as mybir
import concourse.bass_isa as bass_isa
from concourse.bass_utils import run_bass_kernel_spmd

BF16 = mybir.dt.bfloat16
F32 = mybir.dt.float32
F8 = mybir.dt.float8e4
AF = mybir.ActivationFunctionType
ALU = mybir.AluOpType
NP_F8 = ml_dtypes.float8_e4m3

B, T, D = 2, 2048, 768
H, DH, DT = 12, 64, 32
J2 = 2 * DT            # 64, time feature dim; H * J2 == D
NCORES = 8
R = (B * T) // NCORES  # 512 rows per core
P = 128
TTILES = R // P        # 4
KTILES = D // P        # 6
NCH = 384              # projection free-dim chunk (2 chunks of 384 = 768)
SW = J2 * KTILES       # 384, state free width in exchange layout
LN_EPS = 1e-5
REPLICA_GROUPS = [[0, 1, 2, 3], [4, 5, 6, 7]]
DTYPE_MODE = os.environ.get("ABSATT_DTYPE", "fp8")
WIRE_FP8 = os.environ.get("ABSATT_WIRE", "fp8") == "fp8"
WS = 32.0          # fp8 weight scale
LS = 16.0          # fp8 loadT scale
XS = WS * LS       # residual scale in fp8 mode
SS = 256.0         # fp8 exchange-wire scale (state absmax ~0.35 -> ~90)


def _head_perm():
    """perm[kt*128 + g*64 + i] = (kt + 6g)*64 + i  (exchange head order)"""
    perm = np.empty(D, np.int64)
    for kt in range(KTILES):
        for g in range(2):
            h = kt + 6 * g
            perm[kt * P + g * J2: kt * P + g * J2 + J2] = \
                np.arange(h * J2, (h + 1) * J2)
    return perm


# --------------------------------------------------------------------------
# device program
# --------------------------------------------------------------------------

def _build_program(flags, unroll=1, debug_dump=False):
    """flags = (mask_trivial, gb_trivial, bv_zero, bo_zero, qb_uniform, fp8)"""
    mask_trivial, gb_trivial, bv_zero, bo_zero, qb_uniform, fp8 = flags
    MMDT = F8 if fp8 else BF16
    nc = bacc.Bacc("TRN2", target_bir_lowering=False, debug=False,
                   num_devices=NCORES)
    if debug_dump:
        dbg = {
            "d_sa": nc.dram_tensor("d_sa", [P, 5, SW],
                                   F8 if (flags[5] and WIRE_FP8) else BF16,
                                   kind="ExternalOutput").ap(),
            "d_statebf": nc.dram_tensor("d_statebf", [P, SW], BF16,
                                        kind="ExternalOutput").ap(),
            "d_tqs": nc.dram_tensor("d_tqs", [P, KTILES, R], BF16,
                                    kind="ExternalOutput").ap(),
            "d_lt": nc.dram_tensor("d_lt", [P, KTILES, R], BF16,
                                   kind="ExternalOutput").ap(),
            "d_kvs": nc.dram_tensor("d_kvs", [P, TTILES, D], BF16,
                                    kind="ExternalOutput").ap(),
            "d_qsum": nc.dram_tensor("d_qsum", [P, TTILES, H], F32,
                                     kind="ExternalOutput").ap(),
        }

    # ---- I/O ----
    xT_d = nc.dram_tensor("xT", [D, R], MMDT, kind="ExternalInput").ap()
    xr_d = nc.dram_tensor("xr", [R, D], BF16, kind="ExternalInput").ap()
    wqT_d = nc.dram_tensor("wqT", [D, D], MMDT, kind="ExternalInput").ap()
    wkT_d = nc.dram_tensor("wkT", [D, D], MMDT, kind="ExternalInput").ap()
    wvT_d = nc.dram_tensor("wvT", [D, D], MMDT, kind="ExternalInput").ap()
    woT_d = nc.dram_tensor("woT", [D, D], MMDT, kind="ExternalInput").ap()
    tk_d = nc.dram_tensor("tk", [R, D], BF16, kind="ExternalInput").ap()
    tqT_d = nc.dram_tensor("tqT", [D, R], BF16, kind="ExternalInput").ap()
    # rows: [-exp(q_bias), bk, bv, bo]
    bias_d = nc.dram_tensor("biases", [4, D], BF16, kind="ExternalInput").ap()
    e_d = nc.dram_tensor("emat", [H, D], BF16, kind="ExternalInput").ap()
    ones_d = nc.dram_tensor("ones", [P, P], BF16, kind="ExternalInput").ap()
    idf_d = nc.dram_tensor("identf", [P, P], F32, kind="ExternalInput").ap()
    idb_d = nc.dram_tensor("identb", [P, P], BF16, kind="ExternalInput").ap()
    if qb_uniform:
        # per-partition column: -exp(q_bias[0]) / 2 (tanh bias, see Q phase)
        qb_d = nc.dram_tensor("qb_col", [P, 1], F32, kind="ExternalInput").ap()
    if not mask_trivial:
        # cols: [mask, mask/64]
        mask_d = nc.dram_tensor("maskc", [R, 2], F32, kind="ExternalInput").ap()
    if not gb_trivial:
        gb_d = nc.dram_tensor("gb", [2, D], F32, kind="ExternalInput").ap()
    out_d = nc.dram_tensor("out", [R, D], BF16, kind="ExternalOutput").ap()

    rsem = nc.alloc_semaphore("rsem")
    lsem = nc.alloc_semaphore("lsem")
    anchors = {"clears": [], "trig": [], "zop": []}

    with tile.TileContext(nc) as tc:
        with (
            tc.tile_pool(name="per", bufs=1) as per,     # persistent tiles
            tc.tile_pool(name="work", bufs=4) as work,   # per-t-tile working
            tc.tile_pool(name="pj", bufs=int(os.environ.get("ABSATT_PJ", "4")),
                         space="PSUM") as pj_pool,
            tc.tile_pool(name="st", bufs=1, space="PSUM") as st_pool,
            tc.tile_pool(name="auxp", bufs=3, space="PSUM") as aux_pool,
        ):
            # ---- persistent SBUF ----
            wq = per.tile([P, KTILES, D], MMDT, tag="wq")
            wk = per.tile([P, KTILES, D], MMDT, tag="wk")
            wv = per.tile([P, KTILES, D], MMDT, tag="wv")
            wo = per.tile([P, KTILES, D], MMDT, tag="wo")
            xTs = per.tile([P, KTILES, R], MMDT, tag="xTs")
            xrs = per.tile([P, TTILES, D], BF16, tag="xrs")
            tks = per.tile([P, TTILES, D], BF16, tag="tks")
            tqTs = per.tile([P, KTILES, R], BF16, tag="tqTs")
            biass = per.tile([P, 4, D], BF16, tag="biass")
            e_sb = per.tile([H, D], BF16, tag="e_sb")
            ones_l = per.tile([P, P], BF16, tag="ones_l")
            identf = per.tile([P, P], F32, tag="identf")
            identb = per.tile([P, P], BF16, tag="identb")
            eps_sb = per.tile([P, 1], F32, tag="eps_sb")
            scr = per.tile([P, 2], F32, tag="scr")
            qsum_all = per.tile([P, TTILES, H], F32, tag="qsum_all")
            qsumT = per.tile([H, R], BF16, tag="qsumT")
            tqs = per.tile([P, KTILES, R], BF16, tag="tqs")
            loadT = per.tile([P, KTILES, R], MMDT, tag="loadT")
            state_bf = per.tile([P, SW], BF16, tag="state_bf")
            kvs = per.tile([P, TTILES, D], BF16, tag="kvs")
            # exchange buffer: slot 0 = own partial, 1-3 = XOR peers,
            # 4 = zero dummy (forces the sum after the Q phase)
            wire_fp8 = fp8 and WIRE_FP8
            WIRE = F8 if wire_fp8 else BF16
            n_sa = 2 if unroll > 1 else 1
            sa_bufs = [per.tile([P, 5, SW], WIRE, tag=f"sa{i}",
                                name=f"sa{i}")
                       for i in range(n_sa)]
            if qb_uniform:
                qb_sb = per.tile([P, 1], F32, tag="qb_sb")
            if not mask_trivial:
                masks = per.tile([P, TTILES, 2], F32, tag="masks")
            if not gb_trivial:
                gbs = per.tile([P, 2, D], F32, tag="gbs")

            # ---- sem init (spliced to block front post-scheduling) ----
            anchors["clears"].append(nc.gpsimd.sem_clear(rsem))
            anchors["clears"].append(nc.gpsimd.sem_clear(lsem))

            nc.vector.memset(eps_sb[:], LN_EPS * (XS * XS if fp8 else 1.0))
            # prefetch the exp/tanh activation table before the KV phase
            nc.scalar.activation(scr[:, 0:1], eps_sb[:], AF.Exp)

            # ---- input DMAs ----
            # Split per k-tile pair so the first matmuls start as soon as the
            # first chunks land instead of waiting for whole tensors.
            wkT_r = wkT_d.rearrange("(kt p) n -> p kt n", p=P)
            wvT_r = wvT_d.rearrange("(kt p) n -> p kt n", p=P)
            wqT_r = wqT_d.rearrange("(kt p) n -> p kt n", p=P)
            woT_r = woT_d.rearrange("(kt p) n -> p kt n", p=P)
            xT_r = xT_d.rearrange("(kt p) m -> p kt m", p=P)
            tqT_r = tqT_d.rearrange("(kt p) m -> p kt m", p=P)
            nc.sync.dma_start(biass[0:1, :, :], bias_d[None, :, :])
            nc.sync.dma_start(e_sb[:], e_d)
            nc.sync.dma_start(ones_l[:], ones_d)
            nc.sync.dma_start(identb[:], idb_d)
            if qb_uniform:
                nc.sync.dma_start(qb_sb[:], qb_d)
            for k2 in range(KTILES // 2):  # pair granularity = DoubleRow unit
                ksl = bass.ds(2 * k2, 2)
                nc.sync.dma_start(xTs[:, ksl], xT_r[:, ksl])
                nc.sync.dma_start(wk[:, ksl], wkT_r[:, ksl])
            for k2 in range(KTILES // 2):
                nc.sync.dma_start(wv[:, bass.ds(2 * k2, 2)],
                                  wvT_r[:, bass.ds(2 * k2, 2)])
            nc.sync.dma_start(tks[:], tk_d.rearrange("(tt p) f -> p tt f", p=P))
            if not mask_trivial:
                nc.sync.dma_start(masks[:], mask_d.rearrange("(tt p) c -> p tt c", p=P))
            nc.sync.dma_start(wq[:], wqT_r)
            nc.sync.dma_start(identf[:], idf_d)
            nc.sync.dma_start(tqTs[:], tqT_r)
            nc.sync.dma_start(wo[:], woT_r)
            nc.sync.dma_start(xrs[:], xr_d.rearrange("(tt p) f -> p tt f", p=P))
            if not gb_trivial:
                gbs_row = per.tile([1, 2, D], F32, tag="gbs_row")
                nc.sync.dma_start(gbs_row[:], gb_d[None, :, :])
                nc.gpsimd.partition_broadcast(gbs[:], gbs_row[:])

            def proj_psum():
                return [pj_pool.tile([P, 512], F32, tag="pj", name="pj")
                        for _ in range(2)]

            def run_proj(w_tile, tt, bias_idx, psum_tile):
                """psum[:, c, :NCH] = x_tt @ W.T (+ bias row) for both chunks"""
                for c in range(2):
                    nsl = bass.ds(c * NCH, NCH)
                    pc = psum_tile[c]
                    if fp8:
                        for k2 in range(KTILES // 2):
                            nc.tensor.matmul(
                                pc[:, :NCH],
                                xTs[:, 2 * k2:2 * k2 + 2, bass.ts(tt, P)],
                                w_tile[:, 2 * k2:2 * k2 + 2, nsl],
                                start=(k2 == 0),
                                stop=(k2 == KTILES // 2 - 1 and bias_idx is None),
                                perf_mode=mybir.MatmulPerfMode.DoubleRow,
                            )
                    else:
                        for kt in range(KTILES):
                            nc.tensor.matmul(
                                pc[:, :NCH],
                                xTs[:, kt, bass.ts(tt, P)],
                                w_tile[:, kt, nsl],
                                start=(kt == 0),
                                stop=(kt == KTILES - 1 and bias_idx is None),
                            )
                    if bias_idx is not None:
                        nc.tensor.matmul(
                            pc[:, :NCH],
                            ones_l[:],
                            biass[:, bias_idx, nsl],
                            start=False, stop=True,
                        )

            def _iter_body(it):
                sa = sa_bufs[it % n_sa]
                state_ps = st_pool.tile([P, SW], F32, tag="state",
                                        name="state_ps")

                # ================= KV phase (per t-tile) =================
                qt = None
                for tt in range(TTILES):
                    kp = proj_psum()
                    run_proj(wk, tt, 1, kp)
                    # ek = exp(k * mask)  (KTEMP = 1)
                    ek = work.tile([P, D], BF16, tag="ek")
                    denom = work.tile([P, H], F32, tag="denom")
                    for c in range(2):
                        nc.scalar.activation(
                            out=ek[:, bass.ds(c * NCH, NCH)],
                            in_=kp[c][:, :NCH],
                            func=AF.Exp,
                            scale=((1.0 / WS if fp8 else 1.0) if mask_trivial
                                   else masks[:, tt, 0:1]),
                        )
                        nc.vector.reduce_sum(
                            denom[:, bass.ds(c * 6, 6)],
                            ek[:, bass.ds(c * NCH, NCH)].rearrange(
                                "p (h j) -> p h j", j=J2),
                            axis=mybir.AxisListType.X,
                        )
                    if fp8:  # vp carries the W-scale; fold 1/WS via denom
                        nc.vector.tensor_scalar(
                            denom[:], denom[:], WS, None, ALU.mult)
                    recip = work.tile([P, H], F32, tag="recip")
                    nc.vector.reciprocal(recip[:], denom[:])
                    # eks = ek * recip (broadcast over j) = softmax(k)
                    eks = work.tile([P, D], BF16, tag="eks")
                    nc.vector.tensor_tensor(
                        eks[:].rearrange("p (h j) -> p h j", j=J2),
                        ek[:].rearrange("p (h j) -> p h j", j=J2),
                        recip[:, :, None].to_broadcast((P, H, J2)),
                        ALU.mult,
                    )

                    vp = proj_psum()
                    run_proj(wv, tt, None if bv_zero else 2, vp)
                    for c in range(2):
                        nsl = bass.ds(c * NCH, NCH)
                        nc.vector.tensor_tensor(
                            kvs[:, tt, nsl], eks[:, nsl], vp[c][:, :NCH],
                            ALU.mult,
                        )

                # state[64g+j, 64kt+d] += tk_h^T @ kv_h  for h = kt+6g.
                # Each head's accumulation group completes before the next
                # head's start (heads outer, t-tiles inner).
                # high_priority: the exchange trigger is the long pole — the
                # scheduler must not park the state copy behind Q-phase work.
                import contextlib
                hp = (tc.high_priority()
                      if os.environ.get("ABSATT_HP", "1") == "1"
                      else contextlib.nullcontext())
                with hp:
                    for g in range(2):
                        for kt in range(KTILES):
                            h = kt + 6 * g
                            hsl = bass.ds(h * J2, J2)
                            for tt in range(TTILES):
                                nc.tensor.matmul(
                                    state_ps[bass.ds(g * J2, J2),
                                             bass.ds(kt * J2, J2)],
                                    tks[:, tt, hsl],
                                    kvs[:, tt, hsl],
                                    start=(tt == 0), stop=(tt == TTILES - 1),
                                    tile_position=(0, g * J2),
                                )

                    # ======== p2p exchange of the partial state ========
                    nc.scalar.mul(sa[:, 0], state_ps[:],
                                  SS if wire_fp8 else 1.0)
                    for d in (1, 2, 3):
                        nc.gpsimd.remote_dma_broadcast(
                            out_ap=sa[:, d],
                            in_ap=sa[:, 0],
                            remote_sem=rsem,
                            local_sem=lsem,
                            rdests=[(0, d)] + [None] * 7,
                        )
                    anchors["trig"].append(nc.gpsimd.trigger_dma(count=None))

                # ================= Q phase (overlaps the exchange) ======
                tr_full = aux_pool.tile([P, 512], F32, tag="aux", name="tr")
                for tt in range(TTILES):
                    qp = proj_psum()
                    run_proj(wq, tt, None if qb_uniform else 0, qp)
                    # sigmoid(x) = 0.5 + 0.5*tanh(x/2);   qsum = sum_d sigmoid
                    # uniform q_bias folds into the tanh bias: tanh((q-e)/2)
                    qt = work.tile([P, D], BF16, tag="qt")
                    tsum = work.tile([P, H], F32, tag="tsum")
                    for c in range(2):
                        nc.scalar.activation(
                            out=qt[:, bass.ds(c * NCH, NCH)],
                            in_=qp[c][:, :NCH],
                            func=AF.Tanh, scale=0.5 / (WS if fp8 else 1.0),
                            bias=(qb_sb[:] if qb_uniform else 0.0),
                        )
                        nc.vector.reduce_sum(
                            tsum[:, bass.ds(c * 6, 6)],
                            qt[:, bass.ds(c * NCH, NCH)].rearrange(
                                "p (h j) -> p h j", j=J2),
                            axis=mybir.AxisListType.X,
                        )
                    # qsum/DH (*mask) = (tsum*0.5 + 32) * mask / 64
                    if mask_trivial:
                        nc.vector.tensor_scalar(
                            qsum_all[:, tt], tsum[:], 0.5 / DH, 32.0 / DH,
                            ALU.mult, ALU.add,
                        )
                    else:
                        tmp = work.tile([P, H], F32, tag="qtmp")
                        nc.vector.tensor_scalar(
                            tmp[:], tsum[:], 0.5, 32.0, ALU.mult, ALU.add)
                        nc.vector.tensor_scalar(
                            qsum_all[:, tt], tmp[:], masks[:, tt, 1:2], None,
                            ALU.mult,
                        )
                    nc.tensor.transpose(
                        tr_full[:H, bass.ts(tt, P)], qsum_all[:, tt],
                        identf[:])

                # qsumT[h, tt*128+t] via PE transpose (4 slices, one copy)
                nc.scalar.copy(qsumT[:], tr_full[:H, :])

                # tqs[(g,j), kt, t] = tqT * qsum[t, kt+6g] (broadcast via E).
                # The state sum (two halves, gated on rsem via a spliced
                # wait) is interleaved into the chain right after the tqs
                # slices it depends on, so loadT can start on half A while
                # half B and the remaining tqs slices are still in flight.
                def tq_step(kt):
                    qe = aux_pool.tile([P, 512], F32, tag="aux", name="qe")
                    nc.tensor.matmul(
                        qe[:, :R],
                        e_sb[:, bass.ds(kt * P, P)],
                        qsumT[:],
                        start=True, stop=True,
                    )
                    qeb = work.tile([P, R], BF16, tag="qeb")
                    nc.scalar.copy(qeb[:], qe[:, :R])
                    nc.vector.tensor_tensor(
                        tqs[:, kt], tqTs[:, kt], qeb[:], ALU.mult)

                def state_sum(lo, hi):
                    with nc.allow_low_precision(
                            reason="4-term fp8 state sum; bf16 out is plenty"):
                        return nc.vector.reduce_sum(
                            state_bf[:, lo:hi],
                            sa[:, :, lo:hi].rearrange("p s f -> p f s"),
                            axis=mybir.AxisListType.X,
                        )

                tq_step(0)
                # slot 4 <- tqs * 0: a real data dep on the Q phase so the
                # scheduler keeps the sum (and its rsem wait) late.
                nc.vector.tensor_scalar(
                    sa[:, 4], tqs[:, 0, 0:SW], 0.0, None, ALU.mult)
                ra = state_sum(0, SW // 2)
                tq_step(1)
                tq_step(2)
                rb = state_sum(SW // 2, SW)
                anchors["zop"].append([ra, rb])
                for kt in range(3, KTILES):
                    tq_step(kt)

                # prefetch the sqrt table now (after the last tanh; the read
                # of qt pins this after the Q phase so the exp/tanh table is
                # not evicted early)
                nc.scalar.activation(scr[:, 1:2], qt[:, 0:1], AF.Sqrt,
                                     scale=0.0)

                # ================= loadingT =================
                if debug_dump:
                    ltd = per.tile([P, KTILES, R], BF16, tag="ltd")
                for kt in range(KTILES):
                    lt = aux_pool.tile([P, 512], F32, tag="aux", name="lt")
                    ksl = bass.ds(kt * J2, J2)
                    nc.tensor.matmul(
                        lt[0:J2, :R], state_bf[0:J2, ksl], tqs[0:J2, kt, :],
                        start=True, stop=True, tile_position=(0, 0),
                    )
                    nc.tensor.matmul(
                        lt[J2:P, :R], state_bf[J2:P, ksl], tqs[J2:P, kt, :],
                        start=True, stop=True, tile_position=(J2, J2),
                    )
                    if debug_dump:
                        nc.scalar.copy(ltd[:, kt], lt[:, :R])
                    lts = (LS / (SS if wire_fp8 else 1.0)) if fp8 else 1.0
                    # split the psum->fp8 casts between ACT and DVE
                    if kt % 2 == 0:
                        nc.scalar.mul(loadT[:, kt], lt[:, :R], lts)
                    else:
                        nc.vector.tensor_scalar(
                            loadT[:, kt], lt[:, :R], lts, None, ALU.mult)
                if debug_dump:
                    nc.sync.dma_start(dbg["d_sa"], sa[:])
                    nc.sync.dma_start(dbg["d_statebf"], state_bf[:])
                    nc.sync.dma_start(dbg["d_tqs"], tqs[:])
                    nc.sync.dma_start(dbg["d_lt"], ltd[:])
                    nc.sync.dma_start(dbg["d_kvs"], kvs[:])
                    nc.sync.dma_start(dbg["d_qsum"], qsum_all[:])

                # ======== O projection + residual (on PE) + LN ==========
                for tt in range(TTILES):
                    op = proj_psum()
                    for c in range(2):
                        nsl = bass.ds(c * NCH, NCH)
                        oc = op[c]
                        if fp8:
                            for k2 in range(KTILES // 2):
                                nc.tensor.matmul(
                                    oc[:, :NCH],
                                    loadT[:, 2 * k2:2 * k2 + 2, bass.ts(tt, P)],
                                    wo[:, 2 * k2:2 * k2 + 2, nsl],
                                    start=(k2 == 0), stop=False,
                                    perf_mode=mybir.MatmulPerfMode.DoubleRow,
                                )
                        else:
                            for kt in range(KTILES):
                                nc.tensor.matmul(
                                    oc[:, :NCH],
                                    loadT[:, kt, bass.ts(tt, P)],
                                    wo[:, kt, nsl],
                                    start=(kt == 0), stop=False,
                                )
                        if not bo_zero:
                            nc.tensor.matmul(
                                oc[:, :NCH], ones_l[:], biass[:, 3, nsl],
                                start=False, stop=False,
                            )
                        # residual: += x (identity matmul, keeps DVE free)
                        nc.tensor.matmul(
                            oc[:, :NCH], identb[:], xrs[:, tt, nsl],
                            start=False, stop=True,
                        )
                    # layernorm on the PSUM chunks
                    stats = work.tile([P, 2, 6], F32, tag="stats")
                    for c in range(2):
                        nc.vector.bn_stats(stats[:, c], op[c][:, :NCH])
                    mv = work.tile([P, 2], F32, tag="mv")
                    nc.vector.bn_aggr(mv[:], stats[:])
                    std = work.tile([P, 1], F32, tag="std")
                    nc.scalar.activation(std[:], mv[:, 1:2], AF.Sqrt,
                                         bias=eps_sb[:])
                    rstd = work.tile([P, 1], F32, tag="rstd")
                    nc.vector.reciprocal(rstd[:], std[:])
                    nmr = work.tile([P, 1], F32, tag="nmr")
                    nc.vector.tensor_scalar(
                        nmr[:], mv[:, 0:1], rstd[:], -1.0, ALU.mult, ALU.mult)
                    outt = work.tile([P, D], BF16, tag="outt")
                    for c in range(2):
                        nsl = bass.ds(c * NCH, NCH)
                        nc.scalar.activation(
                            outt[:, nsl], op[c][:, :NCH], AF.Identity,
                            scale=rstd[:], bias=nmr[:])
                    if not gb_trivial:
                        nc.vector.tensor_tensor(outt[:], outt[:], gbs[:, 0], ALU.mult)
                        nc.vector.tensor_tensor(outt[:], outt[:], gbs[:, 1], ALU.add)
                    nc.sync.dma_start(
                        out_d.rearrange("(tt p) f -> p tt f", p=P)[:, tt],
                        outt[:])

            for _it in range(unroll):
                _iter_body(_it)

    # ---- post-scheduling: splice in the blocking waits ----
    # (traced inside TileContext these deadlock the tile scheduler's
    # single-core sim, which can't see remote/prelude sem increments)
    def _locate(name):
        for bi, b in enumerate(nc.main_func.blocks):
            for i, ins in enumerate(b.instructions):
                if ins.name == name:
                    return bi, b, i
        raise KeyError(name)

    def wait_before(anchor, engine, sem, val):
        """anchor may be one BassInstruction or a list; the wait lands
        before the earliest of them in the final schedule."""
        w = engine.wait_ge(sem, val)
        _, wb, wi = _locate(w.ins.name)
        del wb.instructions[wi]
        anchor_list = anchor if isinstance(anchor, list) else [anchor]
        best = None
        for a in anchor_list:
            pos = _locate(a.ins.name)
            if best is None or pos[:1] + (pos[2],) < best[:1] + (best[2],):
                best = pos
        best[1].instructions.insert(best[2], w.ins)

    nc._bir_kernel_barrier_sem_replica_groups.extend(
        set(g) for g in REPLICA_GROUPS)
    # Insert the prelude AllGather now (idempotent; compile()'s own call
    # becomes a no-op) so the sem clears can be spliced BEFORE the barrier
    # arrive: peers send only after the AG completes, i.e. after every core
    # has cleared, so no stale value and no lost increment is possible —
    # regardless of whether the runtime re-initializes semaphores between
    # executions.
    nc.insert_bir_kernel_barrier_sem_inc()
    anchors["clears"].append(
        nc.gpsimd.sem_clear(nc._bir_kernel_barrier_sem))
    ag_b, ag_i = None, None
    for b in nc.main_func.blocks:
        for i, ins in enumerate(b.instructions):
            if isinstance(ins, mybir.InstCollectiveCompute) and \
                    "bir_kernel_barrier" in str(ins.outs[0]):
                ag_b, ag_i = b, i
                break
        if ag_b is not None:
            break
    assert ag_b is not None, "prelude AllGather not found"
    for ci in reversed(anchors["clears"]):
        _, cb, cidx = _locate(ci.ins.name)
        del cb.instructions[cidx]
        if cb is ag_b and cidx < ag_i:
            ag_i -= 1
        ag_b.instructions.insert(ag_i, ci.ins)
    for it in range(unroll):
        if it == 0:
            wait_before(anchors["trig"][0], nc.gpsimd,
                        nc._bir_kernel_barrier_sem,
                        nc.bir_kernel_barrier_sem_inc)
        wait_before(anchors["zop"][it], nc.vector, rsem, 6 * (it + 1))

    nc.compile()
    return nc


_PROGRAM_CACHE = {}


def _get_program(flags):
    if flags not in _PROGRAM_CACHE:
        _PROGRAM_CACHE[flags] = _build_program(flags)
    return _PROGRAM_CACHE[flags]


# --------------------------------------------------------------------------
# mirror cost model (test-harness timing only; SPMD-symmetric remote sems)
# --------------------------------------------------------------------------

def make_mirror_cost_model():
    """InstructionCostModel that models remote_dma_broadcast transfers in
    no_exec TimelineSim: my own sends mirror my peers' (SPMD), so the local
    core's remote_sem is bumped after the modeled transfer delay."""
    from concourse.cost_model import InstructionCostModel, SemUpdate, Delay
    from concourse.hw_specs import get_hw_spec
    import concourse.mybir as mb

    hw = get_hw_spec("TRN2")

    class RdmaMirrorCostModel(InstructionCostModel):
        def __init__(self):
            super().__init__(hw)
            self._pending = []

        def visit(self, instruction, sim):
            tl = super().visit(instruction, sim)
            if isinstance(instruction, bass_isa.InstRemoteDMABroadcastDescs):
                self._pending.append(instruction)
            elif (isinstance(instruction, bass_isa.InstTriggerDma)
                  and self._pending):
                # All preps' dests sit at slot 0 (same engine pair), so the
                # transfers serialize: per-prep time uses the slot's engine
                # pair bandwidth, summed across preps.
                total = 0.0
                rsems, lsems = {}, {}
                for pr in self._pending:
                    ndest = len(pr.dests)
                    nreal = sum(1 for dd in pr.dests if dd >= 0)
                    eng_per_dest = 16 // ndest
                    bw = (hw.RDMA_D2D_BANDWIDTH_BYTES_PER_NS_PER_ENGINE
                          * eng_per_dest)
                    total += nreal * pr.free_dim_bytes * 128.0 / bw
                    key = (pr.remote_sem, pr.remote_sem_name)
                    rsems[key] = rsems.get(key, 0) + (16 // ndest) * nreal
                    lu = pr.local_sem_update
                    lkey = (lu.id, lu.ant_name)
                    lsems[lkey] = lsems.get(lkey, 0) + lu.update_value
                self._pending = []
                ev = [Delay(ns=total)]
                for (sid, name), v in rsems.items():
                    ev.append(SemUpdate(updateInfo=mb.SyncUpdate(
                        sync_type="semaphore", id=sid, ant_name=name,
                        update_mode="sem-add-imm", update_value=v)))
                ev.append(Delay(ns=hw.RDMA_D2D_ACK_LATENCY_NS))
                for (sid, name), v in lsems.items():
                    ev.append(SemUpdate(updateInfo=mb.SyncUpdate(
                        sync_type="semaphore", id=sid, ant_name=name,
                        update_mode="sem-add-imm", update_value=v)))
                tl = list(tl)
                tl[0] = list(tl[0]) + ev
            return tl

    return RdmaMirrorCostModel()


# --------------------------------------------------------------------------
# host side
# --------------------------------------------------------------------------

def _time_tensors(time_angle, head_time_delta):
    """time_q/time_k exactly as the reference computes them (f32 angles,
    accurate trig), returned as [T, H*J2] float32 (natural head order)."""
    ta = np.asarray(time_angle, np.float32)
    delta = np.asarray(head_time_delta, np.float32)
    pos = np.arange(T, dtype=np.float32)[:, None, None]
    inv = np.float32(1.0 / np.sqrt(np.float32(DH)))

    def gt(d):
        ang = (pos + d) * ta[None]          # [T, H, DT] fp32 (matches ref)
        a64 = ang.astype(np.float64)
        c, s = np.cos(a64), np.sin(a64)
        return (np.concatenate([c + s, c - s], axis=-1) * np.float64(inv)
                ).astype(np.float32)        # [T, H, J2]

    tq = gt(delta[None, :, None]).reshape(T, H * J2)
    tk = gt(np.float32(0.0)).reshape(T, H * J2)
    return tq, tk


def prepare_inputs(states, attention_mask, Wq, Wk, bk, Wv, bv, Wo, bo, q_bias,
                   time_angle, head_time_delta, ln_gamma, ln_beta):
    f32 = np.float32
    bf16 = ml_dtypes.bfloat16
    states = np.asarray(states, f32)
    mask = np.asarray(attention_mask)
    Wq, Wk, Wv, Wo = (np.asarray(w, f32) for w in (Wq, Wk, Wv, Wo))
    bk, bv, bo, q_bias = (np.asarray(v, f32) for v in (bk, bv, bo, q_bias))
    ln_gamma, ln_beta = np.asarray(ln_gamma, f32), np.asarray(ln_beta, f32)

    mask_trivial = bool(np.all(mask == 1))
    gb_trivial = bool(np.all(ln_gamma == 1.0) and np.all(ln_beta == 0.0))
    bv_zero = bool(np.all(bv == 0.0))
    bo_zero = bool(np.all(bo == 0.0))
    qb_uniform = bool(np.all(q_bias == q_bias[0]))
    fp8 = DTYPE_MODE == "fp8"
    flags = (mask_trivial, gb_trivial, bv_zero, bo_zero, qb_uniform, fp8)

    if fp8:
        mmdt = NP_F8
        ws, xs = np.float32(WS), np.float32(XS)
        bscale = np.array([WS, WS, WS, XS], f32)[:, None]
    else:
        mmdt = bf16
        ws, xs = np.float32(1.0), np.float32(1.0)
        bscale = np.ones((4, 1), f32)

    perm = _head_perm()
    wqT = np.ascontiguousarray(Wq.T * ws).astype(mmdt)
    wkT = np.ascontiguousarray(Wk.T * ws).astype(mmdt)
    wvT = np.ascontiguousarray(Wv.T * ws).astype(mmdt)
    woT = np.ascontiguousarray((Wo.T * ws)[perm]).astype(mmdt)
    biases = (np.stack([-np.exp(q_bias), bk, bv, bo]) * bscale).astype(bf16)

    tq, tk = _time_tensors(time_angle, head_time_delta)
    tq = tq[:, perm]                        # exchange head order

    # E[h, kt*128 + g*64 + j] = 1 iff h == kt + 6g
    emat = np.zeros((H, D), f32)
    for kt in range(KTILES):
        for g in range(2):
            emat[kt + 6 * g, kt * P + g * J2: kt * P + (g + 1) * J2] = 1.0
    ones_l = np.zeros((P, P), f32)
    ones_l[0, :] = 1.0
    identf = np.eye(P, dtype=f32)

    xf = states.reshape(B * T, D)
    maskf = mask.reshape(B * T).astype(f32)

    in_maps = []
    for c in range(NCORES):
        rows = slice(c * R, (c + 1) * R)
        tpos = slice((c % 4) * R, (c % 4) * R + R)
        m = {
            "xT": np.ascontiguousarray(xf[rows].T).astype(mmdt),
            "xr": np.ascontiguousarray(xf[rows] * xs).astype(bf16),
            "wqT": wqT, "wkT": wkT, "wvT": wvT, "woT": woT,
            "tk": np.ascontiguousarray(tk[tpos]).astype(bf16),
            "tqT": np.ascontiguousarray(tq[tpos].T).astype(bf16),
            "biases": biases,
            "emat": emat.astype(bf16),
            "ones": ones_l.astype(bf16),
            "identf": identf,
            "identb": identf.astype(bf16),
        }
        if qb_uniform:
            m["qb_col"] = np.full((P, 1), -np.exp(q_bias[0]) / 2.0, f32)
        if not mask_trivial:
            mc = maskf[rows]
            m["maskc"] = np.stack([mc / (WS if fp8 else 1.0), mc / DH],
                                  axis=1).astype(f32)
        if not gb_trivial:
            m["gb"] = np.stack([ln_gamma, ln_beta]).astype(f32)
        in_maps.append(m)
    return flags, in_maps


def run(inputs, trace=False, trace_kwargs=None):
    flags, in_maps = prepare_inputs(**inputs)
    nc = _get_program(flags)
    res = run_bass_kernel_spmd(
        nc, in_maps, core_ids=list(range(NCORES)),
        trace=trace, **(trace_kwargs or {}))
    full = np.concatenate(
        [np.asarray(res.results[c]["out"]).astype(np.float32)
         for c in range(NCORES)],
        axis=0).reshape(B, T, D)
    return full, res


def kernel(**inputs):
    out, _ = run(inputs)
    return out


if __name__ == "__main__":
    rng = np.random.default_rng(0)
    fake = {
        "states": rng.standard_normal((B, T, D), dtype=np.float32),
        "attention_mask": np.ones((B, T), np.int32),
        "Wq": rng.standard_normal((D, D), dtype=np.float32) * 0.02,
        "Wk": rng.standard_normal((D, D), dtype=np.float32) * 0.02,
        "bk": rng.standard_normal((D,), dtype=np.float32) * 0.02,
        "Wv": rng.standard_normal((D, D), dtype=np.float32) * 0.02,
        "bv": np.zeros((D,), np.float32),
        "Wo": rng.standard_normal((D, D), dtype=np.float32) * 0.02,
        "bo": np.zeros((D,), np.float32),
        "q_bias": np.zeros((D,), np.float32),
        "time_angle": (rng.random((H, DT), dtype=np.float32) ** 10 + 1e-8),
        "head_time_delta": rng.random((H,), dtype=np.float32),
        "ln_gamma": np.ones((D,), np.float32),
        "ln_beta": np.zeros((D,), np.float32),
    }
    out = kernel(**fake)
    print("kernel ran, out shape", out.shape, "finite:",
          np.isfinite(out).all())
